# revision 25
# baseline (speedup 1.0000x reference)
"""EGNN layer (equivariant graph conv) on 8 Trainium2 NeuronCores.

Strategy (graph-partitioned SPMD, no collectives):
  * Host greedily bin-packs the 10000 nodes into 80 "node-tiles"
    (<=126 nodes and <=2048 in-edges per tile); 10 tiles per core.
  * Edges are routed to the tile owning their *target* node, so the
    scatter-add aggregation is a per-tile matmul with a 0/1 selection
    matrix (S_agg).  The target-side feature gather is expressed as a
    matmul expansion with S_exp (one-hot columns); bias and the
    distance term ride along as two extra rows of S_exp.
  * Source-side features are a true gather: one fp16 dma_gather
    (transpose mode) per tile lands node_feat rows directly in the
    feature-major layout the MLP matmuls need; m is moved to edge-major
    for the aggregation matmuls with batched xbar DMA transposes.
  * All MLP activations are kept feature-major [feat(partition), edge
    (free)]; swish runs on the scalar engine straight out of PSUM.

kernel(**inputs) takes the full unsharded inputs and returns
(coord, node_out, vel) exactly like the reference.
"""

import os
import sys

import numpy as np

if "/opt/trn_rl_repo" not in sys.path:
    sys.path.insert(0, "/opt/trn_rl_repo")


import concourse.mybir as mybir
import concourse.tile as tile
from concourse import bacc
from concourse.bass_utils import run_bass_kernel_spmd
from concourse.masks import make_identity

# ---------------------------------------------------------------- constants
N, E, NDIM, IN_DIM, HID = 10000, 160000, 3, 128, 128
P = 128
CORES = 8
TILES = 10          # node-tiles per core
TE = 2048           # edge budget per node-tile (padded)
SUB = TE // P       # 16 subtiles of 128 edges
TN = 126            # real node slots per tile (126=ones row, 127=dist row)
NTILES = CORES * TILES
MEW = 400  # edge-major row width: 800B, 32B-aligned for xbar
F32 = mybir.dt.float32
BF16 = mybir.dt.float16  # 2-byte lane for gather/xbar paths (fp16 > bf16 accuracy)
I32 = mybir.dt.int32
I16 = mybir.dt.int16

LAST_EXEC_NS = None  # set after a traced run


# ---------------------------------------------------------------- device IR
def _emit(nc, io):
    """Emit the per-core program (identical on all cores)."""
    from contextlib import ExitStack

    AF = mybir.ActivationFunctionType
    with tile.TileContext(nc) as tc, ExitStack() as ctx:
        const = ctx.enter_context(tc.tile_pool(name="const", bufs=1))
        pp_mlp = ctx.enter_context(tc.tile_pool(name="pp_mlp", bufs=4, space="PSUM"))
        pp_tr = ctx.enter_context(tc.tile_pool(name="pp_tr", bufs=2, space="PSUM"))
        pp_agg = ctx.enter_context(tc.tile_pool(name="pp_agg", bufs=1, space="PSUM"))
        pp_sm = ctx.enter_context(tc.tile_pool(name="pp_sm", bufs=1, space="PSUM"))
        p_sexp = ctx.enter_context(tc.tile_pool(name="p_sexp", bufs=2))
        p_sagg = ctx.enter_context(tc.tile_pool(name="p_sagg", bufs=2))
        p_rel = ctx.enter_context(tc.tile_pool(name="p_rel", bufs=2))
        p_srcfm = ctx.enter_context(tc.tile_pool(name="p_srcfm", bufs=2))
        p_h1 = ctx.enter_context(tc.tile_pool(name="p_h1", bufs=2))
        p_m3 = ctx.enter_context(tc.tile_pool(name="p_m3", bufs=2))
        p_c1 = ctx.enter_context(tc.tile_pool(name="p_c1", bufs=2))
        p_me = ctx.enter_context(tc.tile_pool(name="p_me", bufs=2))
        p_sm = ctx.enter_context(tc.tile_pool(name="p_sm", bufs=2))
        p_nod = ctx.enter_context(tc.tile_pool(name="p_nod", bufs=2))

        ident = const.tile([P, P], F32, tag="ident")
        make_identity(nc, ident[:])

        # ---- load constants
        def cload(name, shape, dtype=F32):
            t = const.tile(shape, dtype, tag=name)
            nc.sync.dma_start(out=t[:], in_=io[name][:])
            return t

        # critical-path inputs first: gather indices, node features, L1 weights
        idx_sb = cload("idx16", [P, TILES * (TE // 16)], I16)
        w_e1t = cload("w_e1t", [P, HID])
        w_e1s = cload("w_e1s_bf", [P, HID], BF16)
        nfT = cload("nf_slots_t", [P, NDIM * TILES * P])
        w_e2 = cload("w_e2", [P, HID])
        w_c1 = cload("w_c1_bf", [P, HID], BF16)
        w_c2 = cload("w_c2_bf", [P, 1], BF16)
        w_n1a = cload("w_n1a", [P, HID])
        w_n1b = cload("w_n1b", [P, HID])
        w_n2 = cload("w_n2", [P, IN_DIM])
        w_v1 = cload("w_v1", [P, HID])
        w_v2 = cload("w_v2", [P, 1])
        b_e2 = cload("b_e2", [P, 1])
        b_c1 = cload("b_c1", [P, 1])
        b_n1 = cload("b_n1", [P, 1])
        b_n2 = cload("b_n2", [P, 1])
        b_v1 = cload("b_v1", [P, 1])
        b_c2r = cload("b_c2r", [P, 1])
        b_v2r = cload("b_v2r", [P, 1])
        coordsl = cload("coord_slots", [P, TILES * NDIM])
        velsl = cload("velvec_slots", [P, TILES * NDIM])
        degsl = cload("deginv_slots", [P, TILES])

        coord_o = const.tile([P, TILES * NDIM], F32, tag="coord_o")
        vel_o = const.tile([P, TILES * NDIM], F32, tag="vel_o")
        no_sb = const.tile([P, NDIM * TILES * P], F32, tag="no_sb")

        def chunk(d, t):
            return slice((d * TILES + t) * P, (d * TILES + t + 1) * P)

        # ---- A precompute: A[slot, hid] = nf_slot @ We1_t  (+ patch rows)
        A_sb = const.tile([P, NDIM * TILES * P], F32, tag="a_sb")
        nc.sync.dma_start(out=A_sb[TN:P, :], in_=io["a_patch30"][:, :])
        for d in range(NDIM):
            for t in range(TILES):
                pa = pp_tr.tile([P, NDIM, P], F32, tag="tr")
                nc.tensor.matmul(
                    out=pa[:, 0, :], lhsT=nfT[:, chunk(d, t)], rhs=w_e1t[:],
                    start=True, stop=True,
                )
                nc.vector.tensor_copy(out=A_sb[:TN, chunk(d, t)], in_=pa[:TN, 0, :])

        nf_table = io["nf_table"]

        # ---- main edge loop
        for t in range(TILES):
            sexp = p_sexp.tile([P, TE], F32, tag="sexp")
            nc.sync.dma_start(out=sexp[:], in_=io["s_exp"][:, t * TE:(t + 1) * TE])
            sagg = p_sagg.tile([P, TE], BF16, tag="sagg")
            nc.sync.dma_start(out=sagg[:], in_=io["s_agg"][:, t * TE:(t + 1) * TE])
            sagg32 = p_sagg.tile([P, TE], F32, tag="sagg32")
            nc.vector.tensor_copy(out=sagg32[:], in_=sagg[:])
            relt = p_rel.tile([P, SUB, NDIM], F32, tag="rel")
            nc.sync.dma_start(
                out=relt[:],
                in_=io["rel_em"][:, t * SUB * NDIM:(t + 1) * SUB * NDIM],
            )

            # gather src features (bf16), feature-major via xbar transpose
            srcfm = p_srcfm.tile([P, NDIM, TE], BF16, tag="srcfm")
            nc.gpsimd.dma_gather(
                out_ap=srcfm[:],
                in_ap=nf_table[:],
                idxs_ap=idx_sb[:, t * (TE // 16):(t + 1) * (TE // 16)],
                num_idxs=TE,
                num_idxs_reg=TE,
                elem_size=NDIM * IN_DIM,
                transpose=True,
                single_packet=False,
            )

            m3 = p_m3.tile([P, NDIM, TE], BF16, tag="m3")
            me_all = p_me.tile([P, SUB, MEW], BF16, tag="me")
            pcem = pp_sm.tile([P, NDIM * SUB], F32, tag="sm")
            for d in range(NDIM):
                # L1: h1 = silu(We1_s.T @ src + A-expand)   (bias+dist in S)
                h1 = p_h1.tile([P, TE], F32, tag="h1")
                for q in range(TE // 512):
                    qs = slice(q * 512, (q + 1) * 512)
                    pm = pp_mlp.tile([P, 512], F32, tag="mm")
                    nc.tensor.matmul(
                        out=pm[:], lhsT=w_e1s[:], rhs=srcfm[:, d, qs],
                        start=True, stop=False,
                    )
                    nc.tensor.matmul(
                        out=pm[:], lhsT=A_sb[:, chunk(d, t)], rhs=sexp[:, qs],
                        start=False, stop=True,
                    )
                    nc.scalar.activation(out=h1[:, qs], in_=pm[:], func=AF.Silu)
                # L2: m = silu(h1.T @ We2 + be2)
                for q in range(TE // 512):
                    qs = slice(q * 512, (q + 1) * 512)
                    pm = pp_mlp.tile([P, 512], F32, tag="mm")
                    nc.tensor.matmul(
                        out=pm[:], lhsT=w_e2[:], rhs=h1[:, qs],
                        start=True, stop=True,
                    )
                    nc.scalar.activation(
                        out=m3[:, d, qs], in_=pm[:], func=AF.Silu, bias=b_e2[:]
                    )
                # m -> edge-major blocks via xbar DMA transpose
                nc.sync.dma_start_transpose(
                    out=me_all[:, :, d * P:(d + 1) * P], in_=m3[:, d, :]
                )
                # L3: c1 = silu(m.T @ Wc1 + bc1)
                c1 = p_c1.tile([P, TE], BF16, tag="c1")
                for q in range(TE // 512):
                    qs = slice(q * 512, (q + 1) * 512)
                    pm = pp_mlp.tile([P, 512], F32, tag="mm")
                    nc.tensor.matmul(
                        out=pm[:], lhsT=w_c1[:], rhs=m3[:, d, qs],
                        start=True, stop=True,
                    )
                    nc.scalar.activation(
                        out=c1[:, qs], in_=pm[:], func=AF.Silu, bias=b_c1[:]
                    )
                # L4 (edge-major): c[e] = c1_chunk.T @ wc2
                for k in range(SUB):
                    nc.tensor.matmul(
                        out=pcem[:, d * SUB + k:d * SUB + k + 1],
                        lhsT=c1[:, k * P:(k + 1) * P], rhs=w_c2[:],
                        start=True, stop=True,
                    )

            # c + bc2, edge-major [edge, (d,k)]
            cem = p_sm.tile([P, NDIM, SUB], F32)
            nc.vector.tensor_tensor(
                out=cem[:],
                in0=pcem[:].rearrange("p (d k) -> p d k", d=NDIM),
                in1=b_c2r[:].to_broadcast([P, NDIM, SUB]),
                op=mybir.AluOpType.add,
            )

            # aggregate via S_agg (m already edge-major in me_all);
            # coord messages aggregated in f32 for accuracy
            me_cm = p_sm.tile([P, SUB, NDIM], F32, tag="me_cm")
            pagg = pp_agg.tile([P, 512], F32, tag="agg")
            pagg_cm = pp_tr.tile([P, NDIM, P], F32, tag="tr")
            for k in range(SUB):
                nc.vector.tensor_tensor(
                    out=me_cm[:, k, :],
                    in0=relt[:, k, :], in1=cem[:, :, k],
                    op=mybir.AluOpType.mult,
                )
                nc.tensor.matmul(
                    out=pagg[:, 0:NDIM * P],
                    lhsT=sagg[:, k * P:(k + 1) * P],
                    rhs=me_all[:, k, 0:NDIM * P],
                    start=(k == 0), stop=(k == SUB - 1),
                )
                nc.tensor.matmul(
                    out=pagg_cm[:, 0, 0:NDIM],
                    lhsT=sagg32[:, k * P:(k + 1) * P],
                    rhs=me_cm[:, k, :],
                    start=(k == 0), stop=(k == SUB - 1),
                )

            aggsb = p_sm.tile([P, NDIM * P + NDIM], F32)
            nc.vector.tensor_copy(out=aggsb[:, 0:NDIM * P], in_=pagg[:, 0:NDIM * P])
            nc.vector.tensor_copy(
                out=aggsb[:, NDIM * P:NDIM * P + NDIM], in_=pagg_cm[:, 0, 0:NDIM]
            )

            # node-side MLPs for this tile
            psv = pp_sm.tile([P, NDIM * SUB], F32, tag="sm")
            for d in range(NDIM):
                ptr3 = pp_tr.tile([P, NDIM, P], F32, tag="tr")
                nc.tensor.transpose(
                    out=ptr3[:, 0, :], in_=aggsb[:, d * P:(d + 1) * P],
                    identity=ident[:],
                )
                aggT = p_nod.tile([P, P], F32)
                nc.vector.tensor_copy(out=aggT[:], in_=ptr3[:, 0, :])
                pn = pp_tr.tile([P, NDIM, P], F32, tag="tr")
                nc.tensor.matmul(
                    out=pn[:, 0, :], lhsT=w_n1a[:], rhs=nfT[:, chunk(d, t)],
                    start=True, stop=False,
                )
                nc.tensor.matmul(
                    out=pn[:, 0, :], lhsT=w_n1b[:], rhs=aggT[:],
                    start=False, stop=True,
                )
                un = p_nod.tile([P, P], F32)
                nc.scalar.activation(
                    out=un[:], in_=pn[:, 0, :], func=AF.Silu, bias=b_n1[:]
                )
                pn2 = pp_tr.tile([P, NDIM, P], F32, tag="tr")
                nc.tensor.matmul(
                    out=pn2[:, 0, :], lhsT=w_n2[:], rhs=un[:],
                    start=True, stop=True,
                )
                nc.vector.tensor_add(
                    out=no_sb[:, chunk(d, t)], in0=pn2[:, 0, :],
                    in1=nfT[:, chunk(d, t)]
                )
                nc.vector.tensor_tensor(
                    out=no_sb[:, chunk(d, t)], in0=no_sb[:, chunk(d, t)],
                    in1=b_n2[:].to_broadcast([P, P]),
                    op=mybir.AluOpType.add,
                )
                # velocity branch
                pv1 = pp_tr.tile([P, NDIM, P], F32, tag="tr")
                nc.tensor.matmul(
                    out=pv1[:, 0, :], lhsT=w_v1[:], rhs=nfT[:, chunk(d, t)],
                    start=True, stop=True,
                )
                uv = p_nod.tile([P, P], F32)
                nc.scalar.activation(
                    out=uv[:], in_=pv1[:, 0, :], func=AF.Silu, bias=b_v1[:]
                )
                nc.tensor.matmul(
                    out=psv[:, d:d + 1], lhsT=uv[:], rhs=w_v2[:],
                    start=True, stop=True,
                )

            # vel / coord updates for this tile
            ts3 = slice(t * NDIM, (t + 1) * NDIM)
            vsb = p_sm.tile([P, NDIM], F32)
            nc.vector.tensor_tensor(
                out=vsb[:], in0=psv[:, 0:NDIM],
                in1=b_v2r[:].to_broadcast([P, NDIM]),
                op=mybir.AluOpType.add,
            )
            nc.vector.tensor_mul(
                out=vel_o[:, ts3], in0=velsl[:, ts3], in1=vsb[:]
            )
            ctmp = p_sm.tile([P, NDIM], F32)
            nc.vector.tensor_tensor(
                out=ctmp[:], in0=aggsb[:, NDIM * P:NDIM * P + NDIM],
                in1=degsl[:, t:t + 1].to_broadcast([P, NDIM]),
                op=mybir.AluOpType.mult,
            )
            nc.vector.tensor_add(
                out=ctmp[:], in0=ctmp[:], in1=coordsl[:, ts3]
            )
            nc.vector.tensor_add(
                out=coord_o[:, ts3], in0=ctmp[:], in1=vel_o[:, ts3]
            )

        nc.sync.dma_start(out=io["node_out_t"][:], in_=no_sb[:])
        nc.sync.dma_start(out=io["coord_out"][:], in_=coord_o[:])
        nc.sync.dma_start(out=io["vel_out"][:], in_=vel_o[:])


_BUILT = {}


def _build():
    if "nc" in _BUILT:
        return _BUILT["nc"]
    nc = bacc.Bacc(
        "TRN2", target_bir_lowering=False, debug=False, num_devices=CORES
    )
    io = {}

    def din(name, shape, dtype=F32):
        io[name] = nc.dram_tensor(name, shape, dtype, kind="ExternalInput").ap()

    def dout(name, shape, dtype=F32):
        io[name] = nc.dram_tensor(name, shape, dtype, kind="ExternalOutput").ap()

    din("nf_table", [N, NDIM * IN_DIM], BF16)
    din("nf_slots_t", [P, NDIM * TILES * P])
    din("a_patch30", [2, NDIM * TILES * HID])
    din("idx16", [P, TILES * (TE // 16)], I16)
    din("s_exp", [P, TILES * TE])
    din("s_agg", [P, TILES * TE], BF16)
    din("rel_em", [P, TILES * SUB * NDIM])
    din("coord_slots", [P, TILES * NDIM])
    din("velvec_slots", [P, TILES * NDIM])
    din("deginv_slots", [P, TILES])
    io["w_e1s_bf"] = nc.dram_tensor("w_e1s_bf", [P, HID], BF16,
                                    kind="ExternalInput").ap()
    io["w_c1_bf"] = nc.dram_tensor("w_c1_bf", [P, HID], BF16,
                                   kind="ExternalInput").ap()
    io["w_c2_bf"] = nc.dram_tensor("w_c2_bf", [P, 1], BF16,
                                   kind="ExternalInput").ap()
    for nm, shp in [
        ("w_e1t", [P, HID]), ("w_e2", [P, HID]),
        ("w_n1a", [P, HID]),
        ("w_n1b", [P, HID]), ("w_n2", [P, IN_DIM]), ("w_v1", [P, HID]),
        ("w_v2", [P, 1]), ("b_e2", [P, 1]), ("b_c1", [P, 1]),
        ("b_n1", [P, 1]), ("b_n2", [P, 1]), ("b_v1", [P, 1]),
        ("b_c2r", [P, 1]), ("b_v2r", [P, 1]),
    ]:
        din(nm, shp)
    dout("node_out_t", [P, NDIM * TILES * P])
    dout("coord_out", [P, TILES * NDIM])
    dout("vel_out", [P, TILES * NDIM])

    _emit(nc, io)
    nc.compile()
    _BUILT["nc"] = nc
    return nc


# ---------------------------------------------------------------- host prep
def _prepare(node_feat, degree, coordinate, edge_index, velocity_vector,
             We1, be1, We2, be2, Wc1, bc1, Wc2, bc2,
             Wn1, bn1, Wn2, bn2, Wv1, bv1, Wv2, bv2):
    node_feat = np.asarray(node_feat, np.float32)
    degree = np.asarray(degree, np.float32)
    coordinate = np.asarray(coordinate, np.float32)
    velocity_vector = np.asarray(velocity_vector, np.float32)
    ei = np.asarray(edge_index).astype(np.int64)
    src, tgt = ei[0], ei[1]

    rel = coordinate[tgt] - coordinate[src]          # [E, 3]
    dist = np.sum(rel * rel, axis=-1)                # [E]

    indeg = np.bincount(tgt, minlength=N)
    order = np.argsort(tgt, kind="stable")
    starts = np.zeros(N + 1, np.int64)
    np.cumsum(indeg, out=starts[1:])

    # greedy bin-pack nodes into NTILES tiles
    tiles_nodes = []
    cur, cure = [], 0
    for n in range(N):
        if len(cur) == TN or cure + indeg[n] > TE:
            tiles_nodes.append(cur)
            cur, cure = [], 0
        cur.append(n)
        cure += indeg[n]
    if cur:
        tiles_nodes.append(cur)
    assert len(tiles_nodes) <= NTILES, f"bin packing overflow: {len(tiles_nodes)}"
    while len(tiles_nodes) < NTILES:
        tiles_nodes.append([])

    in_maps = []
    slot2node = np.full((CORES, TILES, P), -1, np.int64)

    shared = {}
    shared["nf_table"] = np.ascontiguousarray(
        node_feat.reshape(N, NDIM * IN_DIM)
    ).astype(np.float16)
    We1 = np.asarray(We1, np.float32)
    shared["a_patch30"] = np.tile(
        np.stack([np.asarray(be1, np.float32), We1[2 * IN_DIM]]),
        (1, NDIM * TILES),
    ).astype(np.float32)
    shared["w_e1t"] = np.ascontiguousarray(We1[:IN_DIM])
    shared["w_e1s_bf"] = np.ascontiguousarray(
        We1[IN_DIM:2 * IN_DIM]).astype(np.float16)
    shared["w_e2"] = np.asarray(We2, np.float32)
    shared["w_c1_bf"] = np.asarray(Wc1, np.float32).astype(np.float16)
    shared["w_c2_bf"] = np.asarray(Wc2, np.float32).reshape(HID, 1).astype(np.float16)
    Wn1 = np.asarray(Wn1, np.float32)
    shared["w_n1a"] = np.ascontiguousarray(Wn1[:IN_DIM])
    shared["w_n1b"] = np.ascontiguousarray(Wn1[IN_DIM:])
    shared["w_n2"] = np.asarray(Wn2, np.float32)
    shared["w_v1"] = np.asarray(Wv1, np.float32)
    shared["w_v2"] = np.asarray(Wv2, np.float32).reshape(HID, 1)
    shared["b_e2"] = np.asarray(be2, np.float32).reshape(HID, 1)
    shared["b_c1"] = np.asarray(bc1, np.float32).reshape(HID, 1)
    shared["b_n1"] = np.asarray(bn1, np.float32).reshape(HID, 1)
    shared["b_n2"] = np.asarray(bn2, np.float32).reshape(IN_DIM, 1)
    shared["b_v1"] = np.asarray(bv1, np.float32).reshape(HID, 1)
    shared["b_c2r"] = np.full((P, 1), np.asarray(bc2, np.float32).reshape(-1)[0],
                              np.float32)
    shared["b_v2r"] = np.full((P, 1), np.asarray(bv2, np.float32).reshape(-1)[0],
                              np.float32)

    for c in range(CORES):
        idx16 = np.zeros((P, TILES * (TE // 16)), np.int16)
        s_exp = np.zeros((P, TILES * TE), np.float32)
        s_agg = np.zeros((P, TILES * TE), np.float16)
        rel_em = np.zeros((P, TILES * SUB * NDIM), np.float32)
        nfT = np.zeros((P, NDIM * TILES * P), np.float32)
        coordsl = np.zeros((P, TILES * NDIM), np.float32)
        velsl = np.zeros((P, TILES * NDIM), np.float32)
        degsl = np.zeros((P, TILES), np.float32)

        for t in range(TILES):
            nodes = tiles_nodes[c * TILES + t]
            edge_ids = []
            eslot = []
            for s, n in enumerate(nodes):
                slot2node[c, t, s] = n
                es = order[starts[n]:starts[n + 1]]
                edge_ids.append(es)
                eslot.append(np.full(len(es), s, np.int64))
                nfT[:, (np.arange(NDIM) * TILES + t) * P + s] = node_feat[n].T
                coordsl[s, t * NDIM:(t + 1) * NDIM] = coordinate[n]
                velsl[s, t * NDIM:(t + 1) * NDIM] = velocity_vector[n]
                degsl[s, t] = 1.0 / degree[n]
            if edge_ids:
                edge_ids = np.concatenate(edge_ids)
                eslot = np.concatenate(eslot)
            else:
                edge_ids = np.zeros(0, np.int64)
                eslot = np.zeros(0, np.int64)
            ne = len(edge_ids)
            assert ne <= TE
            j = np.arange(ne)
            kk, pp = j // P, j % P
            # dma_gather idx layout: idx i at [i%16, i//16] within the
            # tile, replicated across all 8 gpsimd cores (partition groups)
            for rep in range(8):
                idx16[rep * 16 + j % 16, t * (TE // 16) + j // 16] = src[edge_ids]
            # S_exp col (t, j): one-hot target slot + ones row + dist row
            s_exp[eslot, t * TE + j] = 1.0
            s_exp[TN, t * TE + j] = 1.0
            s_exp[TN + 1, t * TE + j] = dist[edge_ids]
            # S_agg col (t, k, s), row p
            s_agg[pp, t * TE + kk * P + eslot] = 1.0
            # rel edge-major: row p, col (t, k, d)
            rel_em[pp[:, None], t * SUB * NDIM + kk[:, None] * NDIM +
                   np.arange(NDIM)[None, :]] = rel[edge_ids]

        m = dict(shared)
        m.update(
            idx16=idx16, s_exp=s_exp, s_agg=s_agg, rel_em=rel_em,
            nf_slots_t=nfT, coord_slots=coordsl, velvec_slots=velsl,
            deginv_slots=degsl,
        )
        in_maps.append(m)
    return in_maps, slot2node


def _unshard(results, slot2node):
    coord = np.zeros((N, NDIM), np.float32)
    node_out = np.zeros((N, NDIM, IN_DIM), np.float32)
    vel = np.zeros((N, NDIM), np.float32)
    for c in range(CORES):
        no_t = results[c]["node_out_t"].reshape(P, NDIM, TILES, P)
        co = results[c]["coord_out"].reshape(P, TILES, NDIM)
        ve = results[c]["vel_out"].reshape(P, TILES, NDIM)
        for t in range(TILES):
            s2n = slot2node[c, t]
            valid = np.nonzero(s2n >= 0)[0]
            if len(valid) == 0:
                continue
            nn = s2n[valid]
            node_out[nn] = no_t[:, :, t, valid].transpose(2, 1, 0)
            coord[nn] = co[valid, t, :]
            vel[nn] = ve[valid, t, :]
    return coord, node_out, vel


def kernel(**inputs):
    global LAST_EXEC_NS
    in_maps, slot2node = _prepare(**inputs)
    nc = _build()
    trace = bool(int(os.environ.get("EGNN_TRACE", "0")))
    res = run_bass_kernel_spmd(
        nc, in_maps, core_ids=list(range(CORES)), trace=trace
    )
    LAST_EXEC_NS = res.exec_time_ns
    return _unshard(res.results, slot2node)
